# revision 1
# baseline (speedup 1.0000x reference)
"""CapsuleLayer dynamic-routing kernel for Trainium2 (8 NeuronCores).

Problem: inputs [B=32, I=2048, J=16], W [N=64, I=2048, D=32, J=16], routings=3.
  inputs_hat[b,n,i,d] = sum_j inputs[b,i,j] * W[n,i,d,j]
  3 rounds of routing (softmax over n, weighted sum over i, squash over d).

Strategy: shard the input-capsule axis I across the 8 cores (I_loc=256).
Each core recomputes its ihat shard from W each round (W streamed from HBM
as bf16 hi/lo pairs; ihat never hits DRAM), keeps its b-state [*, n, i_loc]
in SBUF, and the only cross-core data is the [B, N, D] partial sum s,
AllReduced (256 KB) once per round. Output replicated; host takes core 0's.

Matmuls run in bf16 with error compensation: x = xh + xl, W = Wh + Wl
(each bf16); rounds 1-2 accumulate xh*Wh + xh*Wl + xl*Wh in fp32 PSUM
(error ~2^-18). Round 0 uses xh*Wh only (it feeds logits, not the output).

On-chip layout per round, per group of 4 input capsules i:
  PE: col-tiled matmuls (tile_position=(0,32c)), K=j=16, M=b=32,
      Nf=(n,d)=2048 -> PSUM H-tile [128=(4i,32b), 2048=(64n,32d)]
  DVE/ACT: y = sum_d H*v ; b += y ; c = softmax_n(b) ; tmp2 = c*H
  PE: s_psum += selector.T @ tmp2  (folds partition groups AND sums over i)
"""

import sys

for p in ("/opt/trn_rl_repo",):
    if p not in sys.path:
        sys.path.insert(0, p)

import ml_dtypes
import numpy as np

import concourse.bacc as bacc
import concourse.mybir as mybir
import concourse.tile as tile
from concourse.bass_utils import run_bass_kernel_spmd

# problem constants (hardcoded per harness contract)
B, N, I, D, J = 32, 64, 2048, 32, 16
R = 3  # routings
CORES = 8
I_LOC = I // CORES  # 256
ND = N * D  # 2048
EPS = 1e-7

F32 = mybir.dt.float32
BF16 = mybir.dt.bfloat16
FX = mybir.AxisListType.X
ADD = mybir.AluOpType.add
MAX = mybir.AluOpType.max
ACT = mybir.ActivationFunctionType

GROUPS = I_LOC // 4  # 64 groups of 4 capsules per round
NQ = ND // 512  # free-dim quarters per capsule


def _squash_build(nc, vbpool, smalls, kp, s4, eps_ap):
    """s4: [128, 2048] tile holding s (replicated x4 on partition groups).
    Returns vb [128, 2048] = squash(s) broadcast tile (same replication)."""
    sq = smalls.tile([128, N], F32, tag="sq_sq")
    for h in range(2):
        s2 = kp.tile([128, ND // 2], F32, tag="tmp")
        nc.scalar.square(s2[:], s4[:, h * (ND // 2):(h + 1) * (ND // 2)])
        nc.vector.tensor_reduce(
            sq[:, 32 * h:32 * (h + 1)],
            s2[:].rearrange("p (n d) -> p n d", d=D), axis=FX, op=ADD)
    # t = sqrt(sq + eps)
    t = smalls.tile([128, N], F32, tag="sq_t")
    nc.scalar.activation(t[:], sq[:], ACT.Sqrt, bias=eps_ap)
    # q1 = 1 + sq
    q1 = smalls.tile([128, N], F32, tag="sq_q1")
    nc.scalar.activation(q1[:], sq[:], ACT.Identity, bias=1.0)
    den = smalls.tile([128, N], F32, tag="sq_den")
    nc.vector.tensor_mul(den[:], q1[:], t[:])
    rs = smalls.tile([128, N], F32, tag="sq_rs")
    nc.vector.reciprocal(rs[:], den[:])
    scale = smalls.tile([128, N], F32, tag="sq_scale")
    nc.vector.tensor_mul(scale[:], sq[:], rs[:])
    vb = vbpool.tile([128, ND], F32, tag="sq_vb")
    nc.vector.tensor_mul(
        vb[:].rearrange("p (n d) -> p n d", d=D),
        s4[:].rearrange("p (n d) -> p n d", d=D),
        scale[:, :, None].broadcast_to([128, N, D]),
    )
    return vb


def build_kernel():
    nc = bacc.Bacc("TRN2", target_bir_lowering=False, debug=False)

    xth = nc.dram_tensor("xth", [I_LOC * J, B], BF16, kind="ExternalInput")
    xtl = nc.dram_tensor("xtl", [I_LOC * J, B], BF16, kind="ExternalInput")
    wth = nc.dram_tensor("wth", [I_LOC * J, ND], BF16, kind="ExternalInput")
    wtl = nc.dram_tensor("wtl", [I_LOC * J, ND], BF16, kind="ExternalInput")
    out = nc.dram_tensor("out", [B, N, D], F32, kind="ExternalOutput")

    # collective bounce buffers (one pair per round)
    s_in = [nc.dram_tensor(f"s_in{r}", [B, ND], F32) for r in range(R)]
    s_out = [nc.dram_tensor(f"s_out{r}", [B, ND], F32, addr_space="Shared")
             for r in range(R)]

    wth_v = wth[:].rearrange("(i j) f -> j i f", j=J)
    wtl_v = wtl[:].rearrange("(i j) f -> j i f", j=J)

    with tile.TileContext(nc) as tc:
        with (
            tc.tile_pool(name="persist", bufs=1) as pp,
            tc.tile_pool(name="wsbp", bufs=3) as wsbp,
            tc.tile_pool(name="wgp", bufs=6) as wgp,
            tc.tile_pool(name="vbp", bufs=1) as vbp,
            tc.tile_pool(name="work", bufs=3) as kp,
            tc.tile_pool(name="t2p", bufs=6) as t2p,
            tc.tile_pool(name="hsbp", bufs=6) as hsbp,
            tc.tile_pool(name="s4p", bufs=1) as s4p,
            tc.tile_pool(name="pbig", bufs=1) as pbig,
            tc.tile_pool(name="small", bufs=3) as sp,
            tc.tile_pool(name="psum", bufs=2, space="PSUM") as psp,
            tc.tile_pool(name="psumB", bufs=1, space="PSUM") as psB,
        ):
            # ---- resident tiles ----
            # x chunks for round-0 fused einsum: [128=(8i,16j), 32 chunks, B]
            xsbh = pp.tile([128, I_LOC * J // 128, B], BF16, tag="xsbh")
            nc.sync.dma_start(
                xsbh[:], xth[:].rearrange("(k p) b -> p k b", p=128))
            xsbl = pp.tile([128, I_LOC * J // 128, B], BF16, tag="xsbl")
            nc.sync.dma_start(
                xsbl[:], xtl[:].rearrange("(k p) b -> p k b", p=128))
            # x for per-capsule matmuls: [16=j, I_LOC, B] (partitions 0-15)
            xah = pp.tile([16, I_LOC, B], BF16, tag="xah")
            nc.sync.dma_start(xah[:],
                              xth[:].rearrange("(i j) b -> j i b", j=J))
            xal = pp.tile([16, I_LOC, B], BF16, tag="xal")
            nc.sync.dma_start(xal[:],
                              xtl[:].rearrange("(i j) b -> j i b", j=J))

            # routing logits b: [128=(c,b), GROUPS, N]
            bstate = pp.tile([128, GROUPS, N], F32, tag="bstate")
            nc.gpsimd.memset(bstate[:], 0.0)
            eps_t = pp.tile([128, 1], F32, tag="eps")
            nc.gpsimd.memset(eps_t[:], EPS)
            # selector[p, m] = 1.0 if p % 32 == m  (partition-group fold)
            sel_i = pp.tile([128, B], mybir.dt.int32, tag="sel_i")
            nc.gpsimd.iota(sel_i[:], [[1, B]], channel_multiplier=-1)
            nc.vector.tensor_scalar(sel_i[:], sel_i[:], 31, None,
                                    op0=mybir.AluOpType.bitwise_and)
            sel = pp.tile([128, B], F32, tag="sel")
            nc.vector.tensor_scalar(sel[:], sel_i[:], 0, None,
                                    op0=mybir.AluOpType.is_equal)

            # ---------- round 0: c uniform -> s0 = (1/N) sum_i ihat ----------
            ps0 = psB.tile([B, ND], F32, tag="pss")
            n_chunks = I_LOC * J // 128  # 32
            for k in range(n_chunks):
                wsbh = wsbp.tile([128, ND], BF16, tag="wsb")
                nc.sync.dma_start(wsbh[:], wth[k * 128:(k + 1) * 128, :])
                wsbl = wsbp.tile([128, ND], BF16, tag="wsb")
                nc.sync.dma_start(wsbl[:], wtl[k * 128:(k + 1) * 128, :])
                prods0 = [(xsbh, wsbh, 0), (xsbh, wsbl, 1), (xsbl, wsbh, 2)]
                for xs_, ws_, pi in prods0:
                    for q in range(NQ):
                        nc.tensor.matmul(
                            ps0[:, q * 512:(q + 1) * 512],
                            xs_[:, k, :],
                            ws_[:, q * 512:(q + 1) * 512],
                            start=(k == 0 and pi == 0),
                            stop=(k == n_chunks - 1 and pi == 2),
                        )
            s_loc0 = pbig.tile([B, ND], F32, tag="s_loc")
            nc.scalar.mul(s_loc0[:], ps0[:], 1.0 / N)
            nc.sync.dma_start(s_in[0][:], s_loc0[:])
            nc.gpsimd.collective_compute(
                "AllReduce", ADD,
                replica_groups=[list(range(CORES))],
                ins=[s_in[0].ap().opt()], outs=[s_out[0].ap().opt()],
            )
            s4 = s4p.tile([128, ND], F32, tag="s4")
            for g4 in range(4):
                nc.sync.dma_start(s4[g4 * 32:(g4 + 1) * 32, :], s_out[0][:])
            vb = _squash_build(nc, vbp, sp, kp, s4, eps_t[:])

            # ---------- rounds 1, 2 ----------
            HF = ND // 2  # 1024: free-dim half (n 0-31 / n 32-63)
            for r in (1, 2):
                ps_s = psB.tile([B, ND], F32, tag="pss")
                pending = []  # previous group's tmp2 halves (fold delayed)

                def flush_fold(pend, last, _ps=ps_s):
                    g0, t2s = pend
                    for h in range(2):
                        for q in range(2):
                            f0 = h * HF + q * 512
                            nc.tensor.matmul(
                                _ps[:, f0:f0 + 512],
                                sel[:],
                                t2s[h][:, q * 512:(q + 1) * 512],
                                start=(g0 == 0),
                                stop=(last and h == 1 and q == 1),
                                skip_group_check=True,
                            )

                post = []  # groups whose softmax/tmp2 stage is deferred

                def stage_b(g, hsbs):
                    # softmax over n (|b| is O(1): no max-subtraction needed)
                    bsl = bstate[:, g, :]
                    e = sp.tile([128, N], F32, tag="e")
                    se = sp.tile([128, 1], F32, tag="se")
                    nc.scalar.activation(e[:], bsl, ACT.Exp,
                                         accum_out=se[:])
                    rcp = sp.tile([128, 1], F32, tag="rcp")
                    nc.vector.reciprocal(rcp[:], se[:])
                    cg = sp.tile([128, N], F32, tag="cg")
                    nc.vector.tensor_scalar_mul(cg[:], e[:], rcp[:])
                    # tmp2 = c * H  (folded into ps_s two iterations later)
                    pool_mul = (g % 6 != 5)
                    t2s = []
                    for h in range(2):
                        eng = nc.gpsimd if (h == 1 and pool_mul) else nc.vector
                        tmp2 = t2p.tile([128, HF], F32, tag="tmp2")
                        eng.tensor_mul(
                            tmp2[:].rearrange("p (n d) -> p n d", d=D),
                            hsbs[h][:].rearrange("p (n d) -> p n d", d=D),
                            cg[:, 32 * h:32 * (h + 1), None].broadcast_to(
                                [128, 32, D]),
                        )
                        t2s.append(tmp2)
                    pending.append((g, t2s))

                for g in range(GROUPS):
                    # W rows for capsules i = 4g..4g+3
                    wghs, wgls = [], []
                    for u in range(2):
                        wgh_ = wgp.tile([16, 2, ND], BF16, tag="wg")
                        nc.sync.dma_start(
                            wgh_[:], wth_v[:, 4 * g + 2 * u:4 * g + 2 * u + 2, :])
                        wghs.append(wgh_)
                        wgl_ = wgp.tile([16, 2, ND], BF16, tag="wg")
                        nc.sync.dma_start(
                            wgl_[:], wtl_v[:, 4 * g + 2 * u:4 * g + 2 * u + 2, :])
                        wgls.append(wgl_)
                    hsbs = []
                    pg0 = psp.tile([128, HF], F32, tag="pg")
                    pg1 = psp.tile([128, HF], F32, tag="pg")
                    pgs = [pg0, pg1]
                    for c in range(4):
                        i = 4 * g + c
                        wgh, wgl = wghs[c // 2], wgls[c // 2]
                        seq = [(xah, wgh, 0), (xah, wgl, 1), (xal, wgh, 2)]
                        for xa_, wg_, pi in seq:
                            for h in range(2):
                                for q in range(2):
                                    f0 = h * HF + q * 512
                                    nc.tensor.matmul(
                                        pgs[h][32 * c:32 * (c + 1),
                                               q * 512:(q + 1) * 512],
                                        xa_[:, i, :],
                                        wg_[:, c % 2, f0:f0 + 512],
                                        start=(pi == 0), stop=(pi == 2),
                                        tile_position=(0, 32 * c),
                                    )
                    for h in range(2):
                        # stage H half to SBUF on ScalarE; frees PSUM fast
                        hsb = hsbp.tile([128, HF], F32, tag="hsb")
                        nc.scalar.copy(hsb[:], pgs[h][:])
                        hsbs.append(hsb)
                    # fold tmp2 from two stage-B's back
                    if len(pending) >= 2:
                        flush_fold(pending.pop(0), False)
                    # y = sum_d H * v   (h1 muls on GpSimd most groups)
                    pool_mul = (g % 6 != 5)
                    y = sp.tile([128, N], F32, tag="y")
                    for h in range(2):
                        eng = nc.gpsimd if (h == 1 and pool_mul) else nc.vector
                        tmp = kp.tile([128, HF], F32, tag="tmp")
                        eng.tensor_mul(tmp[:], hsbs[h][:],
                                       vb[:, h * HF:(h + 1) * HF])
                        nc.vector.tensor_reduce(
                            y[:, 32 * h:32 * (h + 1)],
                            tmp[:].rearrange("p (n d) -> p n d", d=D),
                            axis=FX, op=ADD)
                    # b += y
                    bsl = bstate[:, g, :]
                    nc.vector.tensor_add(bsl, bsl, y[:])
                    # deferred softmax/tmp2 for the previous group
                    post.append((g, hsbs))
                    if len(post) >= 2:
                        stage_b(*post.pop(0))
                stage_b(*post.pop(0))
                flush_fold(pending.pop(0), False)
                flush_fold(pending.pop(0), False)
                flush_fold(pending.pop(0), True)

                s_loc = pbig.tile([B, ND], F32, tag="s_loc")
                nc.scalar.copy(s_loc[:], ps_s[:])
                nc.sync.dma_start(s_in[r][:], s_loc[:])
                nc.gpsimd.collective_compute(
                    "AllReduce", ADD,
                    replica_groups=[list(range(CORES))],
                    ins=[s_in[r].ap().opt()], outs=[s_out[r].ap().opt()],
                )
                s4 = s4p.tile([128, ND], F32, tag="s4")
                for g4 in range(4):
                    nc.sync.dma_start(s4[g4 * 32:(g4 + 1) * 32, :],
                                      s_out[r][:])
                vb = _squash_build(nc, vbp, sp, kp, s4, eps_t[:])

            # output = squash(s2) = vb rows 0..31
            nc.sync.dma_start(
                out[:].rearrange("b n d -> b (n d)"), vb[0:32, :])

    nc.compile()
    return nc


_NC_CACHE = {}


def _get_nc():
    if "nc" not in _NC_CACHE:
        _NC_CACHE["nc"] = build_kernel()
    return _NC_CACHE["nc"]


def _hi_lo(a):
    hi = a.astype(ml_dtypes.bfloat16)
    lo = (a - hi.astype(np.float32)).astype(ml_dtypes.bfloat16)
    return hi, lo


def _make_in_maps(inputs, W):
    inputs = np.ascontiguousarray(np.asarray(inputs, dtype=np.float32))
    W = np.ascontiguousarray(np.asarray(W, dtype=np.float32))
    assert inputs.shape == (B, I, J) and W.shape == (N, I, D, J)
    in_maps = []
    for c in range(CORES):
        sl = slice(c * I_LOC, (c + 1) * I_LOC)
        # xt: [(i j), b]
        x_t = np.ascontiguousarray(
            inputs[:, sl, :].transpose(1, 2, 0).reshape(I_LOC * J, B))
        # wt: [(i j), (n d)] ; wt[(i,j),(n,d)] = W[n, i, d, j]
        w_t = np.ascontiguousarray(
            W[:, sl, :, :].transpose(1, 3, 0, 2).reshape(I_LOC * J, ND))
        xh, xl = _hi_lo(x_t)
        wh, wl = _hi_lo(w_t)
        in_maps.append({"xth": np.ascontiguousarray(xh),
                        "xtl": np.ascontiguousarray(xl),
                        "wth": np.ascontiguousarray(wh),
                        "wtl": np.ascontiguousarray(wl)})
    return in_maps


def _ensure_ntff_hook():
    """Register the axon NTFF profile hook if the image's antenv lacks it."""
    import types

    try:
        import antenv.axon_hooks  # noqa: F401
        return
    except ImportError:
        pass
    import antenv

    if "/root/.axon_site" not in sys.path:
        sys.path.insert(0, "/root/.axon_site")
    from trn_agent_boot.trn_boot import _ntff_profile_via_ctypes

    hook = {"h": _ntff_profile_via_ctypes("/opt/axon/libaxon_pjrt.so")}
    mod = types.ModuleType("antenv.axon_hooks")
    mod.get_axon_ntff_profile_hook = lambda: hook["h"]
    mod.set_axon_ntff_profile_hook = lambda h: hook.__setitem__("h", h)
    sys.modules["antenv.axon_hooks"] = mod
    antenv.axon_hooks = mod


def run(inputs, W, trace=False):
    nc = _get_nc()
    if trace:
        _ensure_ntff_hook()
        # zero-egress container: skip the artifact upload, keep files local
        import concourse.bass_utils as bu
        bu.upload_artifacts = lambda d: d
    res = run_bass_kernel_spmd(
        nc, _make_in_maps(inputs, W), core_ids=list(range(CORES)),
        trace=trace,
    )
    return res.results[0]["out"].reshape(B, N, D), res


def kernel(inputs, W, routings=R, **_unused):
    assert int(routings) == R
    out, _ = run(inputs, W, trace=False)
    return out



# revision 4
# speedup vs baseline: 2.2572x; 2.2572x over previous
"""CapsuleLayer dynamic-routing kernel for Trainium2 (8 NeuronCores).

Problem: inputs [B=32, I=2048, J=16], W [N=64, I=2048, D=32, J=16], routings=3.
  inputs_hat[b,n,i,d] = sum_j inputs[b,i,j] * W[n,i,d,j]
  3 rounds of routing (softmax over n, weighted sum over i, squash over d).

Strategy: shard the input-capsule axis I across the 8 cores (I_loc=256).
Each core recomputes its ihat shard from W each round (W streamed from HBM
in bf16; ihat never hits DRAM), keeps its b-state [*, n, i_loc] in SBUF,
and the only cross-core data is the [B, N, D] partial sum s, AllReduced
once per round (bf16 for rounds 0-1, fp32 for the output round).

All matmuls are single-product bf16 (output tolerance is loose enough that
hi/lo error compensation is unnecessary). Per group of 4 input capsules i:
  PE:  one K=64 block-diag matmul set streams W once ->
       H PSUM [128=(4i,32b), 2048=(32d,64n)]   (free layout d-outer!)
  SC:  stage H -> SBUF bf16
  DVE: tmpv = H*vb ; y = tree-sum over d (dense contiguous adds) ;
       b += y ; c = softmax_n(b)
  GS:  tmp2 = c*H
  PE:  s_psum += sel.T @ tmp2  (folds partition groups AND sums over i)
The (d,n) free layout makes every tree add a dense step-1 bf16 op (2x DVE
mode) and keeps broadcast operands inner-contiguous.
"""

import sys

for p in ("/opt/trn_rl_repo",):
    if p not in sys.path:
        sys.path.insert(0, p)

import ml_dtypes
import numpy as np

import concourse.bacc as bacc
import concourse.mybir as mybir
import concourse.tile as tile
from concourse.bass_utils import run_bass_kernel_spmd

# problem constants (hardcoded per harness contract)
B, N, I, D, J = 32, 64, 2048, 32, 16
R = 3  # routings
CORES = 8
I_LOC = I // CORES  # 256
ND = N * D  # 2048
EPS = 1e-7

F32 = mybir.dt.float32
BF16 = mybir.dt.bfloat16
FX = mybir.AxisListType.X
ADD = mybir.AluOpType.add
ACT = mybir.ActivationFunctionType

GROUPS = I_LOC // 4  # 64 groups of 4 capsules per round


def _squash_build(nc, vbpool, sp, kp, s4, eps_ap, out_dtype=BF16):
    """s4: [128, 2048] (d,n) tile holding s (replicated x4 on partition
    groups). Returns vb [128, 2048] = squash(s) broadcast tile (bf16)."""
    s2 = kp.tile([128, ND], F32, tag="sq_s2", bufs=1)
    nc.scalar.square(s2[:], s4[:])
    sq = sp.tile([128, N], F32, tag="sq_sq")
    nc.vector.tensor_reduce(
        sq[:], s2[:].rearrange("p (d n) -> p n d", d=D), axis=FX, op=ADD)
    # t = sqrt(sq + eps)
    t = sp.tile([128, N], F32, tag="sq_t")
    nc.scalar.activation(t[:], sq[:], ACT.Sqrt, bias=eps_ap)
    # q1 = 1 + sq
    q1 = sp.tile([128, N], F32, tag="sq_q1")
    nc.scalar.activation(q1[:], sq[:], ACT.Identity, bias=1.0)
    den = sp.tile([128, N], F32, tag="sq_den")
    nc.vector.tensor_mul(den[:], q1[:], t[:])
    rs = sp.tile([128, N], F32, tag="sq_rs")
    nc.vector.reciprocal(rs[:], den[:])
    scale = sp.tile([128, N], F32, tag="sq_scale")
    nc.vector.tensor_mul(scale[:], sq[:], rs[:])
    vb = vbpool.tile([128, ND], out_dtype, tag="sq_vb")
    nc.vector.tensor_mul(
        vb[:].rearrange("p (d n) -> p d n", d=D),
        s4[:].rearrange("p (d n) -> p d n", d=D),
        scale[:, None, :].broadcast_to([128, D, N]),
    )
    return vb


def build_kernel():
    nc = bacc.Bacc("TRN2", target_bir_lowering=False, debug=False)

    # x: [(i j), b] bf16 ; w: [(i j), (d n)] bf16  with w[(i,j),(d,n)] =
    # W[n, i, d, j] (d OUTER, n INNER in the free dim).
    xth = nc.dram_tensor("xth", [I_LOC * J, B], BF16, kind="ExternalInput")
    wth = nc.dram_tensor("wth", [I_LOC * J, ND], BF16, kind="ExternalInput")
    out = nc.dram_tensor("out", [B, N, D], F32, kind="ExternalOutput")

    # collective bounce buffers (one pair per round); bf16 for r<2
    s_in = [nc.dram_tensor(f"s_in{r}", [B, ND], BF16 if r < 2 else F32)
            for r in range(R)]
    s_out = [nc.dram_tensor(f"s_out{r}", [B, ND], BF16 if r < 2 else F32,
                            addr_space="Shared")
             for r in range(R)]

    with tile.TileContext(nc) as tc:
        with (
            tc.tile_pool(name="persist", bufs=1) as pp,
            tc.tile_pool(name="wsbp", bufs=3) as wsbp,   # round-0 W chunks
            tc.tile_pool(name="wgp", bufs=4) as wgp,     # group W tiles
            tc.tile_pool(name="vbp", bufs=2) as vbp,
            tc.tile_pool(name="work", bufs=2) as kp,
            tc.tile_pool(name="t2p", bufs=3) as t2p,     # tmp2 (fold input)
            tc.tile_pool(name="hsbp", bufs=6) as hsbp,   # staged H bf16
            tc.tile_pool(name="tvp", bufs=2) as tvp,     # tmpv + tree
            tc.tile_pool(name="s4p", bufs=2) as s4p,
            tc.tile_pool(name="pbig", bufs=1) as pbig,
            tc.tile_pool(name="small", bufs=3) as sp,
            tc.tile_pool(name="psum", bufs=2, space="PSUM") as psp,
            tc.tile_pool(name="psumB", bufs=1, space="PSUM") as psB,
        ):
            # ---- resident tiles ----
            # round-0 stationary: [128=(8i,16j), 32 chunks, B]
            xsb = pp.tile([128, I_LOC * J // 128, B], BF16, tag="xsb")
            nc.sync.dma_start(
                xsb[:], xth[:].rearrange("(k p) b -> p k b", p=128))
            # block-diag stationary for per-capsule rounds:
            # xblk[16c+j, g, 32c+b] = x[b, 4g+c, j]
            xblk = pp.tile([64, GROUPS, 128], BF16, tag="xblk")
            nc.gpsimd.memset(xblk[:], 0.0)
            xv = xth[:].rearrange("(g c j) b -> c j g b", c=4, j=J)
            for c in range(4):
                nc.sync.dma_start(
                    xblk[16 * c:16 * (c + 1), :, 32 * c:32 * (c + 1)], xv[c])

            # routing logits b: [128=(c,b), GROUPS, N]
            bstate = pp.tile([128, GROUPS, N], F32, tag="bstate")
            nc.gpsimd.memset(bstate[:], 0.0)
            eps_t = pp.tile([128, 1], F32, tag="eps")
            nc.gpsimd.memset(eps_t[:], EPS)
            # selector[p, m] = 1.0 if p % 32 == m  (partition-group fold)
            sel_i = pp.tile([128, B], mybir.dt.int32, tag="sel_i")
            nc.gpsimd.iota(sel_i[:], [[1, B]], channel_multiplier=-1)
            nc.vector.tensor_scalar(sel_i[:], sel_i[:], 31, None,
                                    op0=mybir.AluOpType.bitwise_and)
            sel = pp.tile([128, B], BF16, tag="sel")
            nc.vector.tensor_scalar(sel[:], sel_i[:], 0, None,
                                    op0=mybir.AluOpType.is_equal)

            # ---------- round 0: c uniform -> s0 = (1/N) sum_i ihat ----------
            ps0 = psB.tile([B, ND], F32, tag="pss")
            n_chunks = I_LOC * J // 128  # 32
            for k in range(n_chunks):
                ws = wsbp.tile([128, ND], BF16, tag="wsb")
                nc.sync.dma_start(ws[:], wth[k * 128:(k + 1) * 128, :])
                for q in range(4):
                    nc.tensor.matmul(
                        ps0[:, q * 512:(q + 1) * 512],
                        xsb[:, k, :],
                        ws[:, q * 512:(q + 1) * 512],
                        start=(k == 0),
                        stop=(k == n_chunks - 1),
                    )
            s_loc0 = pbig.tile([B, ND], BF16, tag="s_loc")
            nc.scalar.activation(s_loc0[:], ps0[:], ACT.Copy, scale=1.0 / N)
            nc.sync.dma_start(s_in[0][:], s_loc0[:])
            nc.gpsimd.collective_compute(
                "AllReduce", ADD,
                replica_groups=[list(range(CORES))],
                ins=[s_in[0].ap().opt()], outs=[s_out[0].ap().opt()],
            )
            s4 = s4p.tile([128, ND], BF16, tag="s4")
            for g4 in range(4):
                nc.sync.dma_start(s4[g4 * 32:(g4 + 1) * 32, :], s_out[0][:])
            vb = _squash_build(nc, vbp, sp, kp, s4, eps_t[:])

            # ---------- rounds 1, 2 ----------
            HF = ND // 2  # 1024 free-dim half (d 0-15 / d 16-31)
            for r in (1, 2):
                ps_s = psB.tile([B, ND], F32, tag="pss")
                for g in range(GROUPS):
                    # W rows for capsules i = 4g..4g+3 : [(4i,16j), (d,n)]
                    wg = wgp.tile([64, ND], BF16, tag="wg")
                    nc.sync.dma_start(wg[:], wth[64 * g:64 * (g + 1), :])
                    # H via one block-diag K=64 matmul set (2 PSUM halves)
                    pg0 = psp.tile([128, HF], F32, tag="pg")
                    pg1 = psp.tile([128, HF], F32, tag="pg")
                    pgs = [pg0, pg1]
                    for h in range(2):
                        for q in range(2):
                            f0 = h * HF + q * 512
                            nc.tensor.matmul(
                                pgs[h][:, q * 512:(q + 1) * 512],
                                xblk[:, g, :],
                                wg[:, f0:f0 + 512],
                                start=True, stop=True,
                            )
                    # stage H -> SBUF bf16 (ScalarE; frees PSUM fast)
                    hsb = hsbp.tile([128, ND], BF16, tag="hsb")
                    for h in range(2):
                        nc.scalar.copy(hsb[:, h * HF:(h + 1) * HF], pgs[h][:])
                    # y = sum_d H*vb : mul then dense binary tree over d
                    tmpv = tvp.tile([128, ND], BF16, tag="tmpv")
                    nc.vector.tensor_mul(tmpv[:], hsb[:], vb[:])
                    t16 = tvp.tile([128, HF], BF16, tag="t16")
                    nc.vector.tensor_add(t16[:], tmpv[:, :HF], tmpv[:, HF:])
                    t8 = tvp.tile([128, HF // 2], BF16, tag="t8")
                    nc.vector.tensor_add(t8[:], t16[:, :HF // 2],
                                         t16[:, HF // 2:])
                    t4 = tvp.tile([128, HF // 4], BF16, tag="t4")
                    nc.vector.tensor_add(t4[:], t8[:, :HF // 4],
                                         t8[:, HF // 4:])
                    t2 = tvp.tile([128, HF // 8], BF16, tag="t2")
                    nc.vector.tensor_add(t2[:], t4[:, :HF // 8],
                                         t4[:, HF // 8:])
                    y = sp.tile([128, N], F32, tag="y")
                    nc.vector.tensor_add(y[:], t2[:, :N], t2[:, N:])
                    # b += y ; softmax over n
                    bsl = bstate[:, g, :]
                    nc.vector.tensor_add(bsl, bsl, y[:])
                    e = sp.tile([128, N], BF16, tag="e")
                    se = sp.tile([128, 1], F32, tag="se")
                    nc.scalar.activation(e[:], bsl, ACT.Exp, accum_out=se[:])
                    rcp = sp.tile([128, 1], F32, tag="rcp")
                    nc.vector.reciprocal(rcp[:], se[:])
                    cg = sp.tile([128, N], BF16, tag="cg")
                    nc.vector.tensor_scalar_mul(cg[:], e[:], rcp[:])
                    # tmp2 = c * H  (GpSimd; c broadcast over outer d)
                    tmp2 = t2p.tile([128, ND], BF16, tag="tmp2")
                    nc.gpsimd.tensor_mul(
                        tmp2[:].rearrange("p (d n) -> p d n", d=D),
                        hsb[:].rearrange("p (d n) -> p d n", d=D),
                        cg[:, None, :].broadcast_to([128, D, N]),
                    )
                    # fold into s accumulator
                    for q in range(4):
                        nc.tensor.matmul(
                            ps_s[:, q * 512:(q + 1) * 512],
                            sel[:],
                            tmp2[:, q * 512:(q + 1) * 512],
                            start=(g == 0),
                            stop=(g == GROUPS - 1),
                            skip_group_check=True,
                        )

                s_loc = pbig.tile([B, ND], BF16 if r < 2 else F32,
                                  tag="s_loc" if r < 2 else "s_locf")
                nc.scalar.copy(s_loc[:], ps_s[:])
                nc.sync.dma_start(s_in[r][:], s_loc[:])
                nc.gpsimd.collective_compute(
                    "AllReduce", ADD,
                    replica_groups=[list(range(CORES))],
                    ins=[s_in[r].ap().opt()], outs=[s_out[r].ap().opt()],
                )
                if r < 2:
                    s4 = s4p.tile([128, ND], BF16, tag="s4")
                    for g4 in range(4):
                        nc.sync.dma_start(s4[g4 * 32:(g4 + 1) * 32, :],
                                          s_out[r][:])
                    vb = _squash_build(nc, vbp, sp, kp, s4, eps_t[:])
                else:
                    # final: squash(s2) rows 0..31, permute (d,n)->(n,d), f32
                    s4f = s4p.tile([B, ND], F32, tag="s4f", bufs=1)
                    nc.sync.dma_start(s4f[:], s_out[r][:])
                    s2t = kp.tile([B, ND], F32, tag="sq_s2", bufs=1)
                    nc.scalar.square(s2t[:], s4f[:])
                    sqf = sp.tile([B, N], F32, tag="sqf")
                    nc.vector.tensor_reduce(
                        sqf[:], s2t[:].rearrange("p (d n) -> p n d", d=D),
                        axis=FX, op=ADD)
                    tf = sp.tile([B, N], F32, tag="sq_t")
                    nc.scalar.activation(tf[:], sqf[:], ACT.Sqrt,
                                         bias=eps_t[0:B, :])
                    q1f = sp.tile([B, N], F32, tag="sq_q1")
                    nc.scalar.activation(q1f[:], sqf[:], ACT.Identity,
                                         bias=1.0)
                    denf = sp.tile([B, N], F32, tag="sq_den")
                    nc.vector.tensor_mul(denf[:], q1f[:], tf[:])
                    rsf = sp.tile([B, N], F32, tag="sq_rs")
                    nc.vector.reciprocal(rsf[:], denf[:])
                    scf = sp.tile([B, N], F32, tag="sq_scale")
                    nc.vector.tensor_mul(scf[:], sqf[:], rsf[:])
                    # out32[b, (n,d)] = s4f[b, (d,n) permuted] * scale[b,n]
                    out32 = pbig.tile([B, ND], F32, tag="out32")
                    nc.vector.tensor_mul(
                        out32[:].rearrange("p (n d) -> p n d", d=D),
                        s4f[:].rearrange("p (d n) -> p n d", d=D),
                        scf[:, :, None].broadcast_to([B, N, D]),
                    )
                    nc.sync.dma_start(
                        out[:].rearrange("b n d -> b (n d)"), out32[:])

    nc.compile()
    return nc


_NC_CACHE = {}


def _get_nc():
    if "nc" not in _NC_CACHE:
        _NC_CACHE["nc"] = build_kernel()
    return _NC_CACHE["nc"]


def _make_in_maps(inputs, W):
    inputs = np.ascontiguousarray(np.asarray(inputs, dtype=np.float32))
    W = np.ascontiguousarray(np.asarray(W, dtype=np.float32))
    assert inputs.shape == (B, I, J) and W.shape == (N, I, D, J)
    in_maps = []
    for c in range(CORES):
        sl = slice(c * I_LOC, (c + 1) * I_LOC)
        # xt: [(i j), b]
        x_t = inputs[:, sl, :].transpose(1, 2, 0).reshape(I_LOC * J, B)
        # w2: [(i j), (d n)] ; w2[(i,j),(d,n)] = W[n, i, d, j]
        w_t = W[:, sl, :, :].transpose(1, 3, 2, 0).reshape(I_LOC * J, ND)
        xh = x_t.astype(ml_dtypes.bfloat16)
        wh = w_t.astype(ml_dtypes.bfloat16)
        in_maps.append({"xth": np.ascontiguousarray(xh),
                        "wth": np.ascontiguousarray(wh)})
    return in_maps


def _ensure_ntff_hook():
    """Register the axon NTFF profile hook if the image's antenv lacks it."""
    import types

    try:
        import antenv.axon_hooks  # noqa: F401
        return
    except ImportError:
        pass
    import antenv

    if "/root/.axon_site" not in sys.path:
        sys.path.insert(0, "/root/.axon_site")
    from trn_agent_boot.trn_boot import _ntff_profile_via_ctypes

    hook = {"h": _ntff_profile_via_ctypes("/opt/axon/libaxon_pjrt.so")}
    mod = types.ModuleType("antenv.axon_hooks")
    mod.get_axon_ntff_profile_hook = lambda: hook["h"]
    mod.set_axon_ntff_profile_hook = lambda h: hook.__setitem__("h", h)
    sys.modules["antenv.axon_hooks"] = mod
    antenv.axon_hooks = mod


def run(inputs, W, trace=False):
    nc = _get_nc()
    if trace:
        _ensure_ntff_hook()
        # zero-egress container: skip the artifact upload, keep files local
        import concourse.bass_utils as bu
        bu.upload_artifacts = lambda d: d
    res = run_bass_kernel_spmd(
        nc, _make_in_maps(inputs, W), core_ids=list(range(CORES)),
        trace=trace,
    )
    return res.results[0]["out"].reshape(B, N, D), res


def kernel(inputs, W, routings=R, **_unused):
    assert int(routings) == R
    out, _ = run(inputs, W, trace=False)
    return out


# revision 6
# speedup vs baseline: 2.4773x; 1.0975x over previous
"""CapsuleLayer dynamic-routing kernel for Trainium2 (8 NeuronCores).

Problem: inputs [B=32, I=2048, J=16], W [N=64, I=2048, D=32, J=16], routings=3.
  inputs_hat[b,n,i,d] = sum_j inputs[b,i,j] * W[n,i,d,j]
  3 rounds of routing (softmax over n, weighted sum over i, squash over d).

Strategy: shard the input-capsule axis I across the 8 cores (I_loc=256).
Each core recomputes its ihat shard from W each round (W streamed from HBM
in bf16; ihat never hits DRAM), keeps its b-state [*, n, i_loc] in SBUF,
and the only cross-core data is the [B, N, D] partial sum s, AllReduced
once per round (bf16 for rounds 0-1, fp32 for the output round).

All matmuls are single-product bf16 (output tolerance is loose enough that
hi/lo error compensation is unnecessary). Per group of 4 input capsules i:
  PE:  one K=64 block-diag matmul set streams W once ->
       H PSUM [128=(4i,32b), 2048=(32d,64n)]   (free layout d-outer!)
  SC:  stage H -> SBUF bf16
  DVE: tmpv = H*vb ; y = tree-sum over d (dense contiguous adds) ;
       b += y ; c = softmax_n(b)
  GS:  tmp2 = c*H
  PE:  s_psum += sel.T @ tmp2  (folds partition groups AND sums over i)
The (d,n) free layout makes every tree add a dense step-1 bf16 op (2x DVE
mode) and keeps broadcast operands inner-contiguous.
"""

import sys

for p in ("/opt/trn_rl_repo",):
    if p not in sys.path:
        sys.path.insert(0, p)

import ml_dtypes
import numpy as np

import concourse.bacc as bacc
import concourse.mybir as mybir
import concourse.tile as tile
from concourse.bass_utils import run_bass_kernel_spmd

# problem constants (hardcoded per harness contract)
B, N, I, D, J = 32, 64, 2048, 32, 16
R = 3  # routings
CORES = 8
I_LOC = I // CORES  # 256
ND = N * D  # 2048
EPS = 1e-7

F32 = mybir.dt.float32
BF16 = mybir.dt.bfloat16
FX = mybir.AxisListType.X
ADD = mybir.AluOpType.add
ACT = mybir.ActivationFunctionType

GROUPS = I_LOC // 4  # 64 groups of 4 capsules per round


def _squash_build(nc, vbpool, sp, kp, s4, eps_ap, out_dtype=BF16):
    """s4: [128, 2048] (d,n) tile holding s (replicated x4 on partition
    groups). Returns vb [128, 2048] = squash(s) broadcast tile (bf16)."""
    s2 = kp.tile([128, ND], F32, tag="sq_s2", bufs=1)
    nc.scalar.square(s2[:], s4[:])
    sq = sp.tile([128, N], F32, tag="sq_sq")
    nc.vector.tensor_reduce(
        sq[:], s2[:].rearrange("p (d n) -> p n d", d=D), axis=FX, op=ADD)
    # t = sqrt(sq + eps)
    t = sp.tile([128, N], F32, tag="sq_t")
    nc.scalar.activation(t[:], sq[:], ACT.Sqrt, bias=eps_ap)
    # q1 = 1 + sq
    q1 = sp.tile([128, N], F32, tag="sq_q1")
    nc.scalar.activation(q1[:], sq[:], ACT.Identity, bias=1.0)
    den = sp.tile([128, N], F32, tag="sq_den")
    nc.vector.tensor_mul(den[:], q1[:], t[:])
    rs = sp.tile([128, N], F32, tag="sq_rs")
    nc.vector.reciprocal(rs[:], den[:])
    scale = sp.tile([128, N], F32, tag="sq_scale")
    nc.vector.tensor_mul(scale[:], sq[:], rs[:])
    vb = vbpool.tile([128, ND], out_dtype, tag="sq_vb")
    nc.vector.tensor_mul(
        vb[:].rearrange("p (d n) -> p d n", d=D),
        s4[:].rearrange("p (d n) -> p d n", d=D),
        scale[:, None, :].broadcast_to([128, D, N]),
    )
    return vb


def build_kernel():
    nc = bacc.Bacc("TRN2", target_bir_lowering=False, debug=False)

    # x: [(i j), b] bf16 ; w: [(i j), (d n)] bf16  with w[(i,j),(d,n)] =
    # W[n, i, d, j] (d OUTER, n INNER in the free dim).
    xth = nc.dram_tensor("xth", [I_LOC * J, B], BF16, kind="ExternalInput")
    wth = nc.dram_tensor("wth", [I_LOC * J, ND], BF16, kind="ExternalInput")
    out = nc.dram_tensor("out", [B, N, D], F32, kind="ExternalOutput")

    # collective bounce buffers (one pair per round); bf16 for r<2
    s_in = [nc.dram_tensor(f"s_in{r}", [B, ND], BF16 if r < 2 else F32)
            for r in range(R)]
    s_out = [nc.dram_tensor(f"s_out{r}", [B, ND], BF16 if r < 2 else F32,
                            addr_space="Shared")
             for r in range(R)]

    with tile.TileContext(nc) as tc:
        with (
            tc.tile_pool(name="persist", bufs=1) as pp,
            tc.tile_pool(name="wsbp", bufs=3) as wsbp,   # round-0 W chunks
            tc.tile_pool(name="wgp", bufs=6) as wgp,     # group W tiles
            tc.tile_pool(name="vbp", bufs=2) as vbp,
            tc.tile_pool(name="work", bufs=2) as kp,
            tc.tile_pool(name="t2p", bufs=2) as t2p,     # tmp2 (fold input)
            tc.tile_pool(name="hsbp", bufs=8) as hsbp,   # staged H bf16
            tc.tile_pool(name="tvp", bufs=2) as tvp,     # tmpv + tree
            tc.tile_pool(name="s4p", bufs=2) as s4p,
            tc.tile_pool(name="pbig", bufs=1) as pbig,
            tc.tile_pool(name="small", bufs=3) as sp,
            tc.tile_pool(name="psum", bufs=2, space="PSUM") as psp,
            tc.tile_pool(name="psumB", bufs=1, space="PSUM") as psB,
        ):
            # ---- resident tiles ----
            # round-0 stationary: [128=(8i,16j), 32 chunks, B]
            xsb = pp.tile([128, I_LOC * J // 128, B], BF16, tag="xsb")
            nc.sync.dma_start(
                xsb[:], xth[:].rearrange("(k p) b -> p k b", p=128))
            # block-diag stationary for per-capsule rounds:
            # xblk[16c+j, g, 32c+b] = x[b, 4g+c, j]
            xblk = pp.tile([64, GROUPS, 128], BF16, tag="xblk")
            nc.gpsimd.memset(xblk[:], 0.0)
            xv = xth[:].rearrange("(g c j) b -> c j g b", c=4, j=J)
            for c in range(4):
                nc.sync.dma_start(
                    xblk[16 * c:16 * (c + 1), :, 32 * c:32 * (c + 1)], xv[c])

            # routing logits b: [128=(c,b), GROUPS, N]
            bstate = pp.tile([128, GROUPS, N], F32, tag="bstate")
            nc.gpsimd.memset(bstate[:], 0.0)
            eps_t = pp.tile([128, 1], F32, tag="eps")
            nc.gpsimd.memset(eps_t[:], EPS)
            # selector[p, m] = 1.0 if p % 32 == m  (partition-group fold)
            sel_i = pp.tile([128, B], mybir.dt.int32, tag="sel_i")
            nc.gpsimd.iota(sel_i[:], [[1, B]], channel_multiplier=-1)
            nc.vector.tensor_scalar(sel_i[:], sel_i[:], 31, None,
                                    op0=mybir.AluOpType.bitwise_and)
            sel = pp.tile([128, B], BF16, tag="sel")
            nc.vector.tensor_scalar(sel[:], sel_i[:], 0, None,
                                    op0=mybir.AluOpType.is_equal)

            # ---------- round 0: c uniform -> s0 = (1/N) sum_i ihat ----------
            ps0 = psB.tile([B, ND], F32, tag="pss")
            n_chunks = I_LOC * J // 128  # 32
            for k in range(n_chunks):
                ws = wsbp.tile([128, ND], BF16, tag="wsb")
                nc.sync.dma_start(ws[:], wth[k * 128:(k + 1) * 128, :])
                for q in range(4):
                    nc.tensor.matmul(
                        ps0[:, q * 512:(q + 1) * 512],
                        xsb[:, k, :],
                        ws[:, q * 512:(q + 1) * 512],
                        start=(k == 0),
                        stop=(k == n_chunks - 1),
                    )
            s_loc0 = pbig.tile([B, ND], BF16, tag="s_loc")
            nc.scalar.activation(s_loc0[:], ps0[:], ACT.Copy, scale=1.0 / N)
            nc.sync.dma_start(s_in[0][:], s_loc0[:])
            nc.gpsimd.collective_compute(
                "AllReduce", ADD,
                replica_groups=[list(range(CORES))],
                ins=[s_in[0].ap().opt()], outs=[s_out[0].ap().opt()],
            )
            s4 = s4p.tile([128, ND], BF16, tag="s4")
            for g4 in range(4):
                nc.sync.dma_start(s4[g4 * 32:(g4 + 1) * 32, :], s_out[0][:])
            vb = _squash_build(nc, vbp, sp, kp, s4, eps_t[:])

            # ---------- rounds 1, 2 ----------
            HF = ND // 2  # 1024 free-dim half (d 0-15 / d 16-31)
            for r in (1, 2):
                ps_s = psB.tile([B, ND], F32, tag="pss")
                for g in range(GROUPS):
                    # W rows for capsules i = 4g..4g+3 : [(4i,16j), (d,n)]
                    wg = wgp.tile([64, ND], BF16, tag="wg")
                    nc.sync.dma_start(wg[:], wth[64 * g:64 * (g + 1), :])
                    # H via one block-diag K=64 matmul set (2 PSUM halves)
                    pg0 = psp.tile([128, HF], F32, tag="pg")
                    pg1 = psp.tile([128, HF], F32, tag="pg")
                    pgs = [pg0, pg1]
                    for h in range(2):
                        for q in range(2):
                            f0 = h * HF + q * 512
                            nc.tensor.matmul(
                                pgs[h][:, q * 512:(q + 1) * 512],
                                xblk[:, g, :],
                                wg[:, f0:f0 + 512],
                                start=True, stop=True,
                            )
                    # stage H -> SBUF bf16 (ScalarE; frees PSUM fast)
                    hsb = hsbp.tile([128, ND], BF16, tag="hsb")
                    for h in range(2):
                        nc.scalar.copy(hsb[:, h * HF:(h + 1) * HF], pgs[h][:])
                    # y = sum_d H*vb : mul then dense binary tree over d
                    # (big tree levels on GpSimd, rest on DVE)
                    tmpv = tvp.tile([128, ND], BF16, tag="tmpv")
                    nc.vector.tensor_mul(tmpv[:], hsb[:], vb[:])
                    t16 = tvp.tile([128, HF], BF16, tag="t16")
                    nc.gpsimd.tensor_add(t16[:], tmpv[:, :HF], tmpv[:, HF:])
                    t8 = tvp.tile([128, HF // 2], BF16, tag="t8")
                    nc.gpsimd.tensor_add(t8[:], t16[:, :HF // 2],
                                         t16[:, HF // 2:])
                    t4 = tvp.tile([128, HF // 4], BF16, tag="t4")
                    nc.vector.tensor_add(t4[:], t8[:, :HF // 4],
                                         t8[:, HF // 4:])
                    t2 = tvp.tile([128, HF // 8], BF16, tag="t2")
                    nc.vector.tensor_add(t2[:], t4[:, :HF // 8],
                                         t4[:, HF // 8:])
                    y = sp.tile([128, N], F32, tag="y")
                    nc.vector.tensor_add(y[:], t2[:, :N], t2[:, N:])
                    # b += y ; softmax over n
                    bsl = bstate[:, g, :]
                    nc.vector.tensor_add(bsl, bsl, y[:])
                    e = sp.tile([128, N], BF16, tag="e")
                    se = sp.tile([128, 1], F32, tag="se")
                    nc.scalar.activation(e[:], bsl, ACT.Exp, accum_out=se[:])
                    rcp = sp.tile([128, 1], F32, tag="rcp")
                    nc.vector.reciprocal(rcp[:], se[:])
                    cg = sp.tile([128, N], BF16, tag="cg")
                    nc.vector.tensor_scalar_mul(cg[:], e[:], rcp[:])
                    # tmp2 = c * H  (DVE; c broadcast over outer d)
                    tmp2 = t2p.tile([128, ND], BF16, tag="tmp2")
                    nc.vector.tensor_mul(
                        tmp2[:].rearrange("p (d n) -> p d n", d=D),
                        hsb[:].rearrange("p (d n) -> p d n", d=D),
                        cg[:, None, :].broadcast_to([128, D, N]),
                    )
                    # fold into s accumulator
                    for q in range(4):
                        nc.tensor.matmul(
                            ps_s[:, q * 512:(q + 1) * 512],
                            sel[:],
                            tmp2[:, q * 512:(q + 1) * 512],
                            start=(g == 0),
                            stop=(g == GROUPS - 1),
                            skip_group_check=True,
                        )

                s_loc = pbig.tile([B, ND], BF16 if r < 2 else F32,
                                  tag="s_loc" if r < 2 else "s_locf")
                nc.scalar.copy(s_loc[:], ps_s[:])
                nc.sync.dma_start(s_in[r][:], s_loc[:])
                nc.gpsimd.collective_compute(
                    "AllReduce", ADD,
                    replica_groups=[list(range(CORES))],
                    ins=[s_in[r].ap().opt()], outs=[s_out[r].ap().opt()],
                )
                if r < 2:
                    s4 = s4p.tile([128, ND], BF16, tag="s4")
                    for g4 in range(4):
                        nc.sync.dma_start(s4[g4 * 32:(g4 + 1) * 32, :],
                                          s_out[r][:])
                    vb = _squash_build(nc, vbp, sp, kp, s4, eps_t[:])
                else:
                    # final: squash(s2) rows 0..31, permute (d,n)->(n,d), f32
                    s4f = s4p.tile([B, ND], F32, tag="s4f", bufs=1)
                    nc.sync.dma_start(s4f[:], s_out[r][:])
                    s2t = kp.tile([B, ND], F32, tag="sq_s2", bufs=1)
                    nc.scalar.square(s2t[:], s4f[:])
                    sqf = sp.tile([B, N], F32, tag="sqf")
                    nc.vector.tensor_reduce(
                        sqf[:], s2t[:].rearrange("p (d n) -> p n d", d=D),
                        axis=FX, op=ADD)
                    tf = sp.tile([B, N], F32, tag="sq_t")
                    nc.scalar.activation(tf[:], sqf[:], ACT.Sqrt,
                                         bias=eps_t[0:B, :])
                    q1f = sp.tile([B, N], F32, tag="sq_q1")
                    nc.scalar.activation(q1f[:], sqf[:], ACT.Identity,
                                         bias=1.0)
                    denf = sp.tile([B, N], F32, tag="sq_den")
                    nc.vector.tensor_mul(denf[:], q1f[:], tf[:])
                    rsf = sp.tile([B, N], F32, tag="sq_rs")
                    nc.vector.reciprocal(rsf[:], denf[:])
                    scf = sp.tile([B, N], F32, tag="sq_scale")
                    nc.vector.tensor_mul(scf[:], sqf[:], rsf[:])
                    # out32[b, (n,d)] = s4f[b, (d,n) permuted] * scale[b,n]
                    out32 = pbig.tile([B, ND], F32, tag="out32")
                    nc.vector.tensor_mul(
                        out32[:].rearrange("p (n d) -> p n d", d=D),
                        s4f[:].rearrange("p (d n) -> p n d", d=D),
                        scf[:, :, None].broadcast_to([B, N, D]),
                    )
                    nc.sync.dma_start(
                        out[:].rearrange("b n d -> b (n d)"), out32[:])

    nc.compile()
    return nc


_NC_CACHE = {}


def _get_nc():
    if "nc" not in _NC_CACHE:
        _NC_CACHE["nc"] = build_kernel()
    return _NC_CACHE["nc"]


def _make_in_maps(inputs, W):
    inputs = np.ascontiguousarray(np.asarray(inputs, dtype=np.float32))
    W = np.ascontiguousarray(np.asarray(W, dtype=np.float32))
    assert inputs.shape == (B, I, J) and W.shape == (N, I, D, J)
    in_maps = []
    for c in range(CORES):
        sl = slice(c * I_LOC, (c + 1) * I_LOC)
        # xt: [(i j), b]
        x_t = inputs[:, sl, :].transpose(1, 2, 0).reshape(I_LOC * J, B)
        # w2: [(i j), (d n)] ; w2[(i,j),(d,n)] = W[n, i, d, j]
        w_t = W[:, sl, :, :].transpose(1, 3, 2, 0).reshape(I_LOC * J, ND)
        xh = x_t.astype(ml_dtypes.bfloat16)
        wh = w_t.astype(ml_dtypes.bfloat16)
        in_maps.append({"xth": np.ascontiguousarray(xh),
                        "wth": np.ascontiguousarray(wh)})
    return in_maps


def _ensure_ntff_hook():
    """Register the axon NTFF profile hook if the image's antenv lacks it."""
    import types

    try:
        import antenv.axon_hooks  # noqa: F401
        return
    except ImportError:
        pass
    import antenv

    if "/root/.axon_site" not in sys.path:
        sys.path.insert(0, "/root/.axon_site")
    from trn_agent_boot.trn_boot import _ntff_profile_via_ctypes

    hook = {"h": _ntff_profile_via_ctypes("/opt/axon/libaxon_pjrt.so")}
    mod = types.ModuleType("antenv.axon_hooks")
    mod.get_axon_ntff_profile_hook = lambda: hook["h"]
    mod.set_axon_ntff_profile_hook = lambda h: hook.__setitem__("h", h)
    sys.modules["antenv.axon_hooks"] = mod
    antenv.axon_hooks = mod


def run(inputs, W, trace=False):
    nc = _get_nc()
    if trace:
        _ensure_ntff_hook()
        # zero-egress container: skip the artifact upload, keep files local
        import concourse.bass_utils as bu
        bu.upload_artifacts = lambda d: d
    res = run_bass_kernel_spmd(
        nc, _make_in_maps(inputs, W), core_ids=list(range(CORES)),
        trace=trace,
    )
    return res.results[0]["out"].reshape(B, N, D), res


def kernel(inputs, W, routings=R, **_unused):
    assert int(routings) == R
    out, _ = run(inputs, W, trace=False)
    return out


# revision 9
# speedup vs baseline: 3.3888x; 1.3679x over previous
"""CapsuleLayer dynamic-routing kernel for Trainium2 (8 NeuronCores).

Problem: inputs [B=32, I=2048, J=16], W [N=64, I=2048, D=32, J=16], routings=3.
  inputs_hat[b,n,i,d] = sum_j inputs[b,i,j] * W[n,i,d,j]
  3 rounds of routing (softmax over n, weighted sum over i, squash over d).

Strategy: shard the input-capsule axis I across the 8 cores (I_loc=256).
Each core recomputes its ihat shard from W each round (W streamed from HBM
in bf16, fp8 for round 0; ihat never hits DRAM), keeps its b-state
[*, n, i_loc] in SBUF, and the only cross-core data is the [B, N, D]
partial sum s, AllReduced once per round (bf16 for rounds 0-1, fp32 for
the output round).

Per group of 4 input capsules i (free layout (d,n): d OUTER, n INNER):
  PE:  block-diag K=64 matmuls stream W once ->
       H PSUM [128=(4i,32b), 4x512 quarter tiles]
  SC:  stage H -> SBUF bf16 (hsb)
  DVE: tmpv = hsb*vb (dense bf16 2x mode)
  PE:  t16 = d-halves folded via identity-matmul PSUM accumulation
  SC:  stage t16 -> SBUF bf16
  DVE: t8/t4/t2/y dense tree adds ; b += y ; e = exp(b) (SC) ;
       e_rep8 = e replicated 8x (doubling copies) ; sel' = sel*(1/sum e)
  DVE: tmp2 = hsb * e_rep8 (3D quarters, inner-512 keeps 2x mode)
  PE:  s_psum += sel'.T @ tmp2  (folds softmax denom, partition groups,
       AND the sum over i into one accumulating matmul chain)
GpSimd does no large SBUF ops: concurrent GpSimd/DVE SBUF traffic
serializes both engines on this silicon (measured).
"""

import sys

for p in ("/opt/trn_rl_repo",):
    if p not in sys.path:
        sys.path.insert(0, p)

import ml_dtypes
import numpy as np

import concourse.bacc as bacc
import concourse.mybir as mybir
import concourse.tile as tile
from concourse.bass_utils import run_bass_kernel_spmd

# problem constants (hardcoded per harness contract)
B, N, I, D, J = 32, 64, 2048, 32, 16
R = 3  # routings
CORES = 8
I_LOC = I // CORES  # 256
ND = N * D  # 2048
EPS = 1e-7

F32 = mybir.dt.float32
BF16 = mybir.dt.bfloat16
FP8 = mybir.dt.float8e4
FX = mybir.AxisListType.X
ADD = mybir.AluOpType.add
ACT = mybir.ActivationFunctionType

GROUPS = I_LOC // 4  # 64 groups of 4 capsules per round
HF = ND // 2  # 1024
QF = ND // 4  # 512


def _squash_fast(nc, vbpool, sp, kp, s4, eps_ap):
    """bf16 squash for rounds 0-1: s4 [128, 2048] (d,n) bf16 ->
    vb [128, 2048] bf16.  sq computed via bf16 square + dense tree."""
    s2 = kp.tile([128, ND], BF16, tag="sq_s2", bufs=1)
    nc.scalar.square(s2[:], s4[:])
    u16 = kp.tile([128, HF], BF16, tag="sq_u16", bufs=1)
    nc.vector.tensor_add(u16[:], s2[:, :HF], s2[:, HF:])
    u8 = kp.tile([128, QF], BF16, tag="sq_u8", bufs=1)
    nc.vector.tensor_add(u8[:], u16[:, :QF], u16[:, QF:])
    u4 = kp.tile([128, QF // 2], BF16, tag="sq_u4", bufs=1)
    nc.vector.tensor_add(u4[:], u8[:, :QF // 2], u8[:, QF // 2:])
    u2 = kp.tile([128, QF // 4], BF16, tag="sq_u2", bufs=1)
    nc.vector.tensor_add(u2[:], u4[:, :QF // 4], u4[:, QF // 4:])
    sq = sp.tile([128, N], F32, tag="sq_sq")
    nc.vector.tensor_add(sq[:], u2[:, :N], u2[:, N:])
    # t = sqrt(sq + eps); q1 = 1 + sq
    t = sp.tile([128, N], F32, tag="sq_t")
    nc.scalar.activation(t[:], sq[:], ACT.Sqrt, bias=eps_ap)
    q1 = sp.tile([128, N], F32, tag="sq_q1")
    nc.scalar.activation(q1[:], sq[:], ACT.Identity, bias=1.0)
    den = sp.tile([128, N], F32, tag="sq_den")
    nc.vector.tensor_mul(den[:], q1[:], t[:])
    rs = sp.tile([128, N], F32, tag="sq_rs")
    nc.vector.reciprocal(rs[:], den[:])
    # scale replicated to 512 for the cheap 3D mul
    scr = sp.tile([128, QF], BF16, tag="sq_scr")
    nc.vector.tensor_mul(scr[:, :N], sq[:], rs[:])
    nc.vector.tensor_copy(scr[:, N:2 * N], scr[:, :N])
    nc.vector.tensor_copy(scr[:, 2 * N:4 * N], scr[:, :2 * N])
    nc.vector.tensor_copy(scr[:, 4 * N:], scr[:, :4 * N])
    vb = vbpool.tile([128, ND], BF16, tag="sq_vb")
    nc.vector.tensor_mul(
        vb[:].rearrange("p (q f) -> p q f", q=4),
        s4[:].rearrange("p (q f) -> p q f", q=4),
        scr[:, None, :].broadcast_to([128, 4, QF]),
    )
    return vb


def build_kernel():
    nc = bacc.Bacc("TRN2", target_bir_lowering=False, debug=False)

    # x: [(i j), b] ; w: [(i j), (d n)] with w[(i,j),(d,n)] = W[n, i, d, j]
    # (d OUTER, n INNER in the free dim). fp8 copies for round 0.
    xth = nc.dram_tensor("xth", [I_LOC * J, B], BF16, kind="ExternalInput")
    wth = nc.dram_tensor("wth", [I_LOC * J, ND], BF16, kind="ExternalInput")
    out = nc.dram_tensor("out", [B, N, D], F32, kind="ExternalOutput")

    # collective bounce buffers (one pair per round); bf16 for r<2
    s_in = [nc.dram_tensor(f"s_in{r}", [B, ND], BF16 if r < 2 else F32)
            for r in range(R)]
    s_out = [nc.dram_tensor(f"s_out{r}", [B, ND], BF16 if r < 2 else F32,
                            addr_space="Shared")
             for r in range(R)]

    with tile.TileContext(nc) as tc:
        with (
            tc.tile_pool(name="persist", bufs=1) as pp,
            tc.tile_pool(name="wsbp", bufs=3) as wsbp,   # round-0 W chunks
            tc.tile_pool(name="wgp", bufs=6) as wgp,     # group W tiles
            tc.tile_pool(name="vbp", bufs=2) as vbp,
            tc.tile_pool(name="work", bufs=2) as kp,
            tc.tile_pool(name="t2p", bufs=2) as t2p,     # tmp2 (fold input)
            tc.tile_pool(name="hsbp", bufs=6) as hsbp,   # staged H bf16
            tc.tile_pool(name="tvp", bufs=2) as tvp,     # tmpv + tree
            tc.tile_pool(name="s4p", bufs=2) as s4p,
            tc.tile_pool(name="pbig", bufs=1) as pbig,
            tc.tile_pool(name="small", bufs=3) as sp,
            tc.tile_pool(name="psum", bufs=2, space="PSUM") as psp,
            tc.tile_pool(name="psumT", bufs=2, space="PSUM") as pst,
            tc.tile_pool(name="psumB", bufs=1, space="PSUM") as psB,
        ):
            # ---- resident tiles ----
            # round-0 stationary: [128=(8i,16j), 32 chunks, B]
            xsb = pp.tile([128, I_LOC * J // 128, B], BF16, tag="xsb")
            nc.sync.dma_start(
                xsb[:], xth[:].rearrange("(k p) b -> p k b", p=128))
            # block-diag stationary: xblk[16c+j, g, 32c+b] = x[b, 4g+c, j]
            xblk = pp.tile([64, GROUPS, 128], BF16, tag="xblk")
            nc.gpsimd.memset(xblk[:], 0.0)
            xv = xth[:].rearrange("(g c j) b -> c j g b", c=4, j=J)
            for c in range(4):
                nc.sync.dma_start(
                    xblk[16 * c:16 * (c + 1), :, 32 * c:32 * (c + 1)], xv[c])

            # routing logits b: [128=(c,b), GROUPS, N]
            bstate = pp.tile([128, GROUPS, N], F32, tag="bstate")
            nc.gpsimd.memset(bstate[:], 0.0)
            eps_t = pp.tile([128, 1], F32, tag="eps")
            nc.gpsimd.memset(eps_t[:], EPS)
            # selector[p, m] = 1.0 if p % 32 == m  (partition-group fold)
            sel_i = pp.tile([128, B], mybir.dt.int32, tag="sel_i")
            nc.gpsimd.iota(sel_i[:], [[1, B]], channel_multiplier=-1)
            nc.vector.tensor_scalar(sel_i[:], sel_i[:], 31, None,
                                    op0=mybir.AluOpType.bitwise_and)
            sel = pp.tile([128, B], BF16, tag="sel")
            nc.vector.tensor_scalar(sel[:], sel_i[:], 0, None,
                                    op0=mybir.AluOpType.is_equal)
            # identity stationary for the PE d-halving accumulation
            id_i = pp.tile([128, 128], mybir.dt.int32, tag="id_i")
            nc.gpsimd.iota(id_i[:], [[1, 128]], channel_multiplier=-1)
            ident = pp.tile([128, 128], BF16, tag="ident")
            nc.vector.tensor_scalar(ident[:], id_i[:], 0, None,
                                    op0=mybir.AluOpType.is_equal)

            # ---------- round 0 (fp8): s0 = (1/N) sum_i ihat ----------
            ps0 = psB.tile([B, ND], F32, tag="pss")
            n_chunks = I_LOC * J // 128  # 32
            for k in range(n_chunks):
                ws = wsbp.tile([128, ND], BF16, tag="wsb")
                nc.sync.dma_start(ws[:], wth[k * 128:(k + 1) * 128, :])
                for q in range(4):
                    nc.tensor.matmul(
                        ps0[:, q * QF:(q + 1) * QF],
                        xsb[:, k, :],
                        ws[:, q * QF:(q + 1) * QF],
                        start=(k == 0),
                        stop=(k == n_chunks - 1),
                    )
            s_loc0 = pbig.tile([B, ND], BF16, tag="s_loc")
            nc.scalar.activation(s_loc0[:], ps0[:], ACT.Copy, scale=1.0 / N)
            nc.sync.dma_start(s_in[0][:], s_loc0[:])
            nc.gpsimd.collective_compute(
                "AllReduce", ADD,
                replica_groups=[list(range(CORES))],
                ins=[s_in[0].ap().opt()], outs=[s_out[0].ap().opt()],
            )
            s4 = s4p.tile([128, ND], BF16, tag="s4")
            for g4 in range(4):
                nc.sync.dma_start(s4[g4 * 32:(g4 + 1) * 32, :], s_out[0][:])
            vb = _squash_fast(nc, vbp, sp, kp, s4, eps_t[:])

            # ---------- rounds 1, 2 ----------
            for r in (1, 2):
                ps_s = psB.tile([B, ND], F32, tag="pss")
                for g in range(GROUPS):
                    # W rows for capsules i = 4g..4g+3 : [(4i,16j), (d,n)]
                    wg = wgp.tile([64, ND], BF16, tag="wg")
                    nc.sync.dma_start(wg[:], wth[64 * g:64 * (g + 1), :])
                    # H via block-diag K=64 matmuls -> 4 PSUM quarters
                    pgs = []
                    hsb = hsbp.tile([128, ND], BF16, tag="hsb")
                    for q in range(4):
                        pg = psp.tile([128, QF], F32, tag="pg")
                        nc.tensor.matmul(
                            pg[:], xblk[:, g, :],
                            wg[:, q * QF:(q + 1) * QF],
                            start=True, stop=True,
                        )
                        # stage quarter -> SBUF bf16 (ScalarE)
                        nc.scalar.copy(hsb[:, q * QF:(q + 1) * QF], pg[:])
                        pgs.append(pg)
                    # tmpv = H*vb (dense bf16, 2x DVE mode)
                    tmpv = tvp.tile([128, ND], BF16, tag="tmpv")
                    nc.vector.tensor_mul(tmpv[:], hsb[:], vb[:])
                    # t8[k,n] = sum_q tmpv[8q+k, n] via one 4-way
                    # identity-matmul PSUM accumulation (any d-grouping is
                    # valid -- the tree sums all d eventually)
                    pt = pst.tile([128, QF], F32, tag="pt")
                    for q in range(4):
                        nc.tensor.matmul(pt[:], ident[:],
                                         tmpv[:, q * QF:(q + 1) * QF],
                                         start=(q == 0), stop=(q == 3),
                                         skip_group_check=True)
                    t8 = tvp.tile([128, QF], BF16, tag="t8")
                    nc.scalar.copy(t8[:], pt[:])
                    t4 = tvp.tile([128, QF // 2], BF16, tag="t4")
                    nc.vector.tensor_add(t4[:], t8[:, :QF // 2],
                                         t8[:, QF // 2:])
                    t2 = tvp.tile([128, QF // 4], BF16, tag="t2")
                    nc.vector.tensor_add(t2[:], t4[:, :QF // 4],
                                         t4[:, QF // 4:])
                    y = sp.tile([128, N], F32, tag="y")
                    nc.vector.tensor_add(y[:], t2[:, :N], t2[:, N:])
                    # b += y ; softmax pieces
                    bsl = bstate[:, g, :]
                    nc.vector.tensor_add(bsl, bsl, y[:])
                    er = sp.tile([128, QF], BF16, tag="er")
                    se = sp.tile([128, 1], F32, tag="se")
                    nc.scalar.activation(er[:, :N], bsl, ACT.Exp,
                                         accum_out=se[:])
                    nc.vector.tensor_copy(er[:, N:2 * N], er[:, :N])
                    nc.vector.tensor_copy(er[:, 2 * N:4 * N], er[:, :2 * N])
                    nc.vector.tensor_copy(er[:, 4 * N:], er[:, :4 * N])
                    rcp = sp.tile([128, 1], F32, tag="rcp")
                    nc.vector.reciprocal(rcp[:], se[:])
                    # fold stationary: sel' = sel * (1/sum e)  (r folded in)
                    selr = sp.tile([128, B], BF16, tag="selr")
                    nc.vector.tensor_scalar_mul(selr[:], sel[:], rcp[:])
                    # tmp2 = e * H  (3D quarters; inner 512 keeps 2x mode)
                    tmp2 = t2p.tile([128, ND], BF16, tag="tmp2")
                    nc.vector.tensor_mul(
                        tmp2[:].rearrange("p (q f) -> p q f", q=4),
                        hsb[:].rearrange("p (q f) -> p q f", q=4),
                        er[:, None, :].broadcast_to([128, 4, QF]),
                    )
                    # fold into s accumulator
                    for q in range(4):
                        nc.tensor.matmul(
                            ps_s[:, q * QF:(q + 1) * QF],
                            selr[:],
                            tmp2[:, q * QF:(q + 1) * QF],
                            start=(g == 0),
                            stop=(g == GROUPS - 1),
                            skip_group_check=True,
                        )

                s_loc = pbig.tile([B, ND], BF16 if r < 2 else F32,
                                  tag="s_loc" if r < 2 else "s_locf")
                nc.scalar.copy(s_loc[:], ps_s[:])
                nc.sync.dma_start(s_in[r][:], s_loc[:])
                nc.gpsimd.collective_compute(
                    "AllReduce", ADD,
                    replica_groups=[list(range(CORES))],
                    ins=[s_in[r].ap().opt()], outs=[s_out[r].ap().opt()],
                )
                if r < 2:
                    s4 = s4p.tile([128, ND], BF16, tag="s4")
                    for g4 in range(4):
                        nc.sync.dma_start(s4[g4 * 32:(g4 + 1) * 32, :],
                                          s_out[r][:])
                    vb = _squash_fast(nc, vbp, sp, kp, s4, eps_t[:])
                else:
                    # final: squash(s2) rows 0..31, permute (d,n)->(n,d), f32
                    s4f = s4p.tile([B, ND], F32, tag="s4f", bufs=1)
                    nc.sync.dma_start(s4f[:], s_out[r][:])
                    s2t = kp.tile([B, ND], F32, tag="sq_s2f", bufs=1)
                    nc.scalar.square(s2t[:], s4f[:])
                    sqf = sp.tile([B, N], F32, tag="sqf")
                    nc.vector.tensor_reduce(
                        sqf[:], s2t[:].rearrange("p (d n) -> p n d", d=D),
                        axis=FX, op=ADD)
                    tf = sp.tile([B, N], F32, tag="sq_t")
                    nc.scalar.activation(tf[:], sqf[:], ACT.Sqrt,
                                         bias=eps_t[0:B, :])
                    q1f = sp.tile([B, N], F32, tag="sq_q1")
                    nc.scalar.activation(q1f[:], sqf[:], ACT.Identity,
                                         bias=1.0)
                    denf = sp.tile([B, N], F32, tag="sq_den")
                    nc.vector.tensor_mul(denf[:], q1f[:], tf[:])
                    rsf = sp.tile([B, N], F32, tag="sq_rs")
                    nc.vector.reciprocal(rsf[:], denf[:])
                    scf = sp.tile([B, N], F32, tag="sq_scale")
                    nc.vector.tensor_mul(scf[:], sqf[:], rsf[:])
                    # out32[b, (n,d)] = s4f[b, (d,n) permuted] * scale[b,n]
                    out32 = pbig.tile([B, ND], F32, tag="out32")
                    nc.vector.tensor_mul(
                        out32[:].rearrange("p (n d) -> p n d", d=D),
                        s4f[:].rearrange("p (d n) -> p n d", d=D),
                        scf[:, :, None].broadcast_to([B, N, D]),
                    )
                    nc.sync.dma_start(
                        out[:].rearrange("b n d -> b (n d)"), out32[:])

    nc.compile()
    return nc


_NC_CACHE = {}


def _get_nc():
    if "nc" not in _NC_CACHE:
        _NC_CACHE["nc"] = build_kernel()
    return _NC_CACHE["nc"]


def _make_in_maps(inputs, W):
    inputs = np.ascontiguousarray(np.asarray(inputs, dtype=np.float32))
    W = np.ascontiguousarray(np.asarray(W, dtype=np.float32))
    assert inputs.shape == (B, I, J) and W.shape == (N, I, D, J)
    in_maps = []
    for c in range(CORES):
        sl = slice(c * I_LOC, (c + 1) * I_LOC)
        # xt: [(i j), b]
        x_t = inputs[:, sl, :].transpose(1, 2, 0).reshape(I_LOC * J, B)
        # w2: [(i j), (d n)] ; w2[(i,j),(d,n)] = W[n, i, d, j]
        w_t = W[:, sl, :, :].transpose(1, 3, 2, 0).reshape(I_LOC * J, ND)
        in_maps.append({
            "xth": np.ascontiguousarray(x_t.astype(ml_dtypes.bfloat16)),
            "wth": np.ascontiguousarray(w_t.astype(ml_dtypes.bfloat16)),
        })
    return in_maps


def _ensure_ntff_hook():
    """Register the axon NTFF profile hook if the image's antenv lacks it."""
    import types

    try:
        import antenv.axon_hooks  # noqa: F401
        return
    except ImportError:
        pass
    import antenv

    if "/root/.axon_site" not in sys.path:
        sys.path.insert(0, "/root/.axon_site")
    from trn_agent_boot.trn_boot import _ntff_profile_via_ctypes

    hook = {"h": _ntff_profile_via_ctypes("/opt/axon/libaxon_pjrt.so")}
    mod = types.ModuleType("antenv.axon_hooks")
    mod.get_axon_ntff_profile_hook = lambda: hook["h"]
    mod.set_axon_ntff_profile_hook = lambda h: hook.__setitem__("h", h)
    sys.modules["antenv.axon_hooks"] = mod
    antenv.axon_hooks = mod


def run(inputs, W, trace=False):
    nc = _get_nc()
    if trace:
        _ensure_ntff_hook()
        # zero-egress container: skip the artifact upload, keep files local
        import concourse.bass_utils as bu
        bu.upload_artifacts = lambda d: d
    res = run_bass_kernel_spmd(
        nc, _make_in_maps(inputs, W), core_ids=list(range(CORES)),
        trace=trace,
    )
    return res.results[0]["out"].reshape(B, N, D), res


def kernel(inputs, W, routings=R, **_unused):
    assert int(routings) == R
    out, _ = run(inputs, W, trace=False)
    return out
